# revision 33
# baseline (speedup 1.0000x reference)
"""Trainium2 Bass kernel for nn_CAPMemory (camera-aware proxy memory loss).

Strategy (8 NeuronCores, SPMD, no collectives):
  - Shard the 64000x256 proxy table over P: core k owns labels
    [1000k, 1000(k+1)), all 8 cameras. Per-core column layout is CAM-MAJOR
    in the ROW-PERMUTATION camera order (slab s holds camera order[s]), one
    1024-col slab per camera (1000 real + 24 zero-pad). Slabs are grouped in
    PAIRS sharing one 4-bank PSUM tile, so a single drain instruction can
    read 2000 real columns with one fixed-overhead charge. Because rows are
    permuted so camera groups are contiguous in the same order, each row
    tile's exp cameras form a consecutive slab run, leaving the non-exp
    slabs as aligned pairs.
  - Matmuls run in fp8(e4m3) DoubleRow mode: operands laid out [128, 2, free]
    so one matmul contracts all K=256 at 2 MACs/cell/cycle. Centers are
    pre-scaled by 16 on the host so their entries (~N(0,1/256)) sit in e4m3's
    normal range; the 1/16 rides in the host post-scale and the exp scale.
    feats are transposed/quantized on the host (fT input); row norms arrive
    as the sc20 input. Centers SBUF is double-buffered so the fp8 DMA of the
    next iteration hides under compute.
  - PSUM drain per pair-tile, unit kinds chosen statically to balance ACT
    and DVE busy time (exact cost-model constants):
      exp  : per-slab ACT exp (scale=sc20) -> bf16 image half + accum_out
             (intra denominator). Monotone, so the image top-8 are the
             chunk candidates (exp domain). Not pairable (accum per slab).
      cp2  : ONE ACT copy over both slabs [128,2,1000] -> 2000-wide bf16
             image (raw domain).
      dir2 : ONE DVE InstMax over both slabs (exact top-8 of the 2000).
      cp1/dir1: single-slab fallbacks for orphan slabs next to an exp slab.
    Images are folded on DVE by pairwise tensor_max (2x bf16) down to <=256
    stripe maxima and finished with one InstMax into the unit's candidate
    slot.
  - Candidates: top-8 per unit chunk; host merges 8 cores' candidates,
    removes positives by value-matching, takes top-50, and recomputes rows
    whose per-chunk 8th-largest exceeds the merged t50 exactly (fallback).
  - Host merge: intra logsumexp = log(sum_k srow_k); positives in f64;
    per-camera means as in the reference.
"""

import os
import sys
import functools

sys.path.insert(0, "/opt/trn_rl_repo")

import numpy as np

from concourse import bacc, mybir
from concourse.tile import TileContext

F32 = mybir.dt.float32
BF16 = mybir.dt.bfloat16
FP8 = mybir.dt.float8e4
NP_FP8 = mybir.dt.np(FP8)
NP_BF16 = mybir.dt.np(BF16)

N = 512          # batch
D = 256          # feature dim
L = 8000         # labels
C = 8            # cameras
NCORES = 8
L_LOCAL = 1000   # labels per core
RT = 4           # row tiles of 128
SLABW = 1024     # padded columns per camera slab (1000 + 24 pad)
PL = 8 * SLABW   # padded per-core columns (8192)
SW = 1000        # real slab width (one camera's columns)
INV_T = 20.0     # 1 / temperature
K = 50           # hard negatives
LW = 0.5         # inter-cam loss weight
CEN_SCALE = 16.0 # host pre-scale on centers (keeps fp8 in normal range)
CAND_PER_S = 8
SLABS = C
NSLOT = 8                     # candidate slots per (rt, core)
CAND = NSLOT * CAND_PER_S     # 64 candidate values per row-tile per core

# experiment knobs
MM = os.environ.get("V2_MM", "fp8dr")            # fp8dr|bf16
FOLDS_TGT = int(os.environ.get("V2_FOLDS_TGT", "256"))  # fold down to <= this
M1BUFS = int(os.environ.get("V2_M1BUFS", "6"))   # scr/fold tile ring depth
DEFER = int(os.environ.get("V2_DEFER", "2"))     # units to defer fold chains by
CP2_OVR = os.environ.get("V3_CP2", "4")          # dir2 pair count override
PLAN_MODE = os.environ.get("V3_MODE", "dir2")    # dir2|mix2
POOLSUM = os.environ.get("V3_POOLSUM", "0") == "1"  # intra sums on gpsimd

# cost-model constants (ns) used by the static ACT/DVE balance
_ACT_EXP = 1205.0    # single-slab exp with accum read
_ACT_EXP_NOACC = 1018.0  # exp without accum (POOLSUM mode)
_ACT_CP2 = 1852.0    # paired copy [128,2,1000]
_ACT_CP1 = 1018.0    # single copy
_DVE_DIR2 = 2208.0   # paired InstMax from PSUM
_DVE_DIR1 = 1167.0   # single InstMax from PSUM
_DVE_F2K = 1414.0    # fold chain + InstMax for a 2000-wide bf16 image
_DVE_F1K = 833.0     # fold chain + InstMax for a 1000-wide bf16 image
_ACT_BUBBLE = 773.0  # ACT refill bubble per pure-DVE pair drain


def _pair_order(sizes):
    """Order cameras so as few camera blocks as possible cross a 128-row
    tile boundary: every crossing costs one extra intra-exp instruction.
    8! is tiny, so search exhaustively."""
    from itertools import permutations

    sizes = [int(s) for s in sizes]
    best, best_cross = None, None
    for perm in permutations(range(C)):
        acc = cross = 0
        for c in perm:
            lo = acc
            acc += sizes[c]
            cross += (acc - 1) // 128 - lo // 128 if sizes[c] else 0
        if best_cross is None or cross < best_cross:
            best, best_cross = perm, cross
            if cross == 0:
                break
    return list(best)


def _units(tile_cams):
    """Static drain plan shared by device build and host decode.

    Slab s holds camera order[s]; tile_cams is given in SLAB indices here
    (i.e. already mapped through the order). Returns units[rt] = list of
      (kind, slabs, slot) with kind in {exp2, exp1, cp2, cp1s, cp1, dir1}
    where cp1s is a pair of single copies sharing one 2000 image (slabs may
    be non-adjacent). slot is the candidate slot index.

    Pure-DVE pair drains (dir2) are avoided: with the 2-deep PSUM ring a
    pair with no ACT work exposes the next refill (~770ns) as an ACT
    bubble. Instead DVE PSUM work comes from 'mix2' pairs = one slab
    dir1 (DVE) + one slab cp1 (ACT), so every pair keeps ACT busy while
    its partner tile refills.
    """
    free_pairs = []   # (rt, k)
    orphans = []      # (rt, s) slabs next to an exp1 in their pair
    n_exp_i = 0
    n_f2k = 0         # exp2 fold images
    n_f1k = 0         # exp1 fold images
    for rt in range(RT):
        E = set(tile_cams[rt])
        for k in range(SLABS // 2):
            a, b = 2 * k, 2 * k + 1
            ina, inb = a in E, b in E
            if ina and inb:
                n_exp_i += 2
                n_f2k += 1
            elif ina or inb:
                e, o = (a, b) if ina else (b, a)
                n_exp_i += 1
                n_f1k += 1
                orphans.append((rt, o))
            else:
                free_pairs.append((rt, k))

    n_dir1_orph = len(orphans)  # orphans default to dir1
    exp_cost = _ACT_EXP_NOACC if POOLSUM else _ACT_EXP

    P = len(free_pairs)
    by_rt_free = {}
    for rt, k in free_pairs:
        by_rt_free.setdefault(rt, []).append(k)

    def _distribute(m, role):
        """Round-robin `role` across rts (latest free pair of each rt
        first); returns {(rt,k): role|'cp2'} and per-rt count."""
        roles = {fp: "cp2" for fp in free_pairs}
        cnt = {rt: 0 for rt in range(RT)}
        avail = {rt: list(reversed(ks)) for rt, ks in by_rt_free.items()}
        left = m
        for rt in (1, 2, 3, 0) * 4:
            if left <= 0:
                break
            ks = avail.get(rt)
            if ks:
                roles[(rt, ks.pop(0))] = role
                cnt[rt] += 1
                left -= 1
        return roles, cnt

    best_m, best_t = 0, None
    for m in range(P + 1):
        _, cnt = _distribute(m, "x")
        if PLAN_MODE == "mix2":
            n_sh = sum(c // 2 for c in cnt.values())       # shared cp1s
            n_single = sum(c % 2 for c in cnt.values())    # standalone cp1
            act = n_exp_i * exp_cost + (P - m) * _ACT_CP2 + m * _ACT_CP1
            dve = (
                (n_f2k + (P - m) + n_sh) * _DVE_F2K
                + (n_f1k + n_single) * _DVE_F1K
                + (m + n_dir1_orph) * _DVE_DIR1
            )
        else:  # dir2: m pure-DVE pair drains; each exposes an ACT bubble
            act = (
                n_exp_i * exp_cost + (P - m) * _ACT_CP2 + m * _ACT_BUBBLE
            )
            dve = (
                (n_f2k + (P - m)) * _DVE_F2K
                + n_f1k * _DVE_F1K
                + m * _DVE_DIR2
                + n_dir1_orph * _DVE_DIR1
            )
        t = max(act, dve)
        if best_t is None or t < best_t:
            best_t, best_m = t, m
    m = int(CP2_OVR) if CP2_OVR else best_m
    role_name = "mix2" if PLAN_MODE == "mix2" else "dir2"
    roles, _ = _distribute(m, role_name)

    out = []
    for rt in range(RT):
        units = []
        E = set(tile_cams[rt])
        cp1_halves = []
        for k in range(SLABS // 2):
            a, b = 2 * k, 2 * k + 1
            ina, inb = a in E, b in E
            if ina and inb:
                units.append(["exp2", (a, b)])
            elif ina or inb:
                e, o = (a, b) if ina else (b, a)
                units.append(["exp1", (e,)])
                units.append(["dir1", (o,)])
            elif roles[(rt, k)] == "cp2":
                units.append(["cp2", (a, b)])
            elif roles[(rt, k)] == "dir2":
                units.append(["dir2", (a, b)])
            else:  # mix2: even slab -> dir1 (DVE), odd slab -> cp1 (ACT)
                units.append(["dir1", (a,)])
                cp1_halves.append(b)
        i = 0
        while i + 1 < len(cp1_halves):
            units.append(["cp1s", (cp1_halves[i], cp1_halves[i + 1])])
            i += 2
        if i < len(cp1_halves):
            units.append(["cp1", (cp1_halves[i],)])
        final = []
        for slot, (kind, slabs) in enumerate(units):
            final.append((kind, tuple(slabs), slot))
        assert len(final) <= NSLOT, final
        out.append(final)
    return out


@functools.lru_cache(maxsize=8)
def _build_program(tile_cams, repeats=1):
    nc = bacc.Bacc(None, target_bir_lowering=False, num_swdge_queues=4)

    mm_dt = FP8 if MM == "fp8dr" else BF16
    cenT = nc.dram_tensor("cenT", [128, 2, PL], mm_dt, kind="ExternalInput")
    fTd = nc.dram_tensor("fT", [RT, 128, 2, 128], mm_dt, kind="ExternalInput")
    sc20d = nc.dram_tensor("sc20", [128, RT], F32, kind="ExternalInput")
    candd = nc.dram_tensor("cand", [RT, 128, CAND], F32, kind="ExternalOutput")
    srowd = nc.dram_tensor("srow", [RT, 128, C], F32, kind="ExternalOutput")

    with TileContext(nc) as tc:
        with (
            tc.tile_pool(name="cen", bufs=2) as cenp,
            tc.tile_pool(name="ftp", bufs=2) as ftp,
            tc.tile_pool(name="m1p", bufs=M1BUFS) as m1p,
            tc.tile_pool(name="smallp", bufs=2) as smallp,
            tc.tile_pool(name="outp", bufs=2) as outp,
            tc.tile_pool(name="psum", bufs=2, space="PSUM") as psump,
        ):
            # fold chains carry across iteration bodies (flushed with a
            # fixed defer in global pair order); final flush after the
            # last body
            state = {"pending": [], "base": 0, "last_pool": 1}
            for _rep in range(repeats):
                _kernel_body(nc, tc, cenp, ftp, m1p, smallp, outp, psump,
                             cenT, fTd, sc20d, candd, srowd, tile_cams,
                             state, warm=(_rep == 0))
                state["base"] += RT * (SLABS // 2)
            for _, tile_, co_, w_ in state["pending"]:
                _fold_and_max(nc, m1p, co_, tile_, w_)

    nc.compile()
    return nc


def _emit_exp(nc, smallp, dst, cols, sc20_sb, rt, accum_ap):
    """ACT exp into a bf16 image half; intra sum via ACT accum_out, or (in
    POOLSUM mode) via a gpsimd tensor_scalar pass over the image."""
    ActF = mybir.ActivationFunctionType
    if not POOLSUM:
        nc.scalar.activation(
            dst, cols, ActF.Exp,
            scale=sc20_sb[:, rt : rt + 1],
            accum_out=accum_ap,
        )
        return
    nc.scalar.activation(dst, cols, ActF.Exp,
                         scale=sc20_sb[:, rt : rt + 1])
    dummy = smallp.tile([128, SW], BF16, name="psdummy", bufs=2)
    nc.gpsimd.tensor_scalar(
        dummy[:, :], dst, 1.0, None, mybir.AluOpType.mult,
        accum_out=accum_ap,
    )


def _fold_and_max(nc, m1p, co, img, w):
    """DVE: pairwise tensor_max folds (2x bf16 mode) down to <=FOLDS_TGT
    stripe maxima, then InstMax top-8."""
    cur = img
    while w > FOLDS_TGT and w % 2 == 0:
        half = w // 2
        nxt = m1p.tile([128, half], BF16, name="fold")
        nc.vector.tensor_max(nxt[:, :], cur[:, 0:half], cur[:, half : 2 * half])
        cur, w = nxt, half
    nc.vector.max(co, cur[:, 0:w])


def _kernel_body(nc, tc, cenp, ftp, m1p, smallp, outp, psump,
                 cenT, fTd, sc20d, candd, srowd, tile_cams, state,
                 warm=True):
    ActF = mybir.ActivationFunctionType
    mm_dt = FP8 if MM == "fp8dr" else BF16
    plan = _units(tile_cams)

    # small transfers first; warm the Exp LUT in ACT's idle window
    # (first body only — the table stays resident across repeats)
    sc20_sb = smallp.tile([128, RT], F32, name="sc20", bufs=2)
    nc.sync.dma_start(out=sc20_sb[:, :], in_=sc20d[:, :])
    if warm:
        warm_t = smallp.tile([128, 1], F32, name="warm", bufs=2)
        nc.scalar.activation(warm_t[:, 0:1], sc20_sb[:, 0:1], ActF.Exp)

    fTs = []
    for rt in range(RT):
        fT = ftp.tile([128, 2, 128], mm_dt, name=f"fT{rt}")
        nc.sync.dma_start(out=fT[:, :, :], in_=fTd[rt])
        fTs.append(fT)

    # centers: one DMA per 2-slab group (a matmul then waits on a single
    # completion sem); alternate queues so transfers overlap
    # inputs on the sync/vector HWDGE queues (outputs use gpsimd SWDGE;
    # inputs must not share a queue with outputs or a waiting output
    # blocks them). One DMA per 2-slab group: a matmul waits one sem.
    cen_sb = cenp.tile([128, 2, PL], mm_dt, name="cen")
    for g in range(4):
        s = slice(g * 2 * SLABW, (g + 1) * 2 * SLABW)
        nc.sync.dma_start(out=cen_sb[:, :, s], in_=cenT[:, :, s])

    cand_ts = [
        outp.tile([128, CAND], F32, name=f"cand{rt}", bufs=2) for rt in range(RT)
    ]
    s_ts = [
        smallp.tile([128, C], F32, name=f"s_t{rt}", bufs=2) for rt in range(RT)
    ]

    pending_folds = state["pending"]

    def _flush_folds(upto):
        while pending_folds and pending_folds[0][0] <= upto - DEFER:
            _, tile_, co_, w_ = pending_folds.pop(0)
            _fold_and_max(nc, m1p, co_, tile_, w_)

    for rt in range(RT):
        units = plan[rt]

        def _cand_slot(slot):
            return cand_ts[rt][:, slot * CAND_PER_S : (slot + 1) * CAND_PER_S]

        # units indexed by slab; pair-wide units (exp2/cp2) fire once
        unit_of = {}
        for u in units:
            kind, slabs, slot = u
            for s in slabs:
                unit_of[s] = u

        # shared-image state for cp1s units, keyed by unit id
        cp1s_state = {}

        for kpair in range(SLABS // 2):
            seq = state["base"] + rt * (SLABS // 2) + kpair
            _flush_folds(seq)
            ps = psump.tile([128, 4, 512], F32, name="ps")
            for sub in range(2):
                s = 2 * kpair + sub
                for mk in range(2):
                    lo = s * SLABW + mk * 512
                    if MM == "fp8dr":
                        nc.tensor.matmul(
                            ps[:, 2 * sub + mk, :], fTs[rt][:, :, :],
                            cen_sb[:, :, lo : lo + 512],
                            start=True, stop=True,
                            perf_mode=mybir.MatmulPerfMode.DoubleRow,
                        )
                    else:
                        nc.tensor.matmul(
                            ps[:, 2 * sub + mk, :], fTs[rt][:, 0, :],
                            cen_sb[:, 0, lo : lo + 512],
                            start=True, stop=False,
                        )
                        nc.tensor.matmul(
                            ps[:, 2 * sub + mk, :], fTs[rt][:, 1, :],
                            cen_sb[:, 1, lo : lo + 512],
                            start=False, stop=True,
                        )

            flat = ps.rearrange("p a b -> p (a b)")     # [128, 2048]
            cols0 = flat[:, 0:SW]                        # slab 2k real cols
            cols1 = flat[:, SLABW : SLABW + SW]          # slab 2k+1 real cols

            # pair-wide units fire once; per-slab units fire per slab.
            # DVE dir1 drains are emitted FIRST so they sit ahead of this
            # pair's fold work in the DVE queue (they release PSUM).
            pair_us = []
            seen = set()
            for s in (2 * kpair, 2 * kpair + 1):
                u = unit_of[s]
                if id(u) not in seen:
                    seen.add(id(u))
                    pair_us.append(u)
            pair_us.sort(key=lambda u: u[0] != "dir1")

            for u in pair_us:
                kind, slabs, slot = u
                if kind == "dir1":
                    for s in slabs:
                        if s // 2 == kpair:
                            cols = cols0 if s % 2 == 0 else cols1
                            nc.vector.max(_cand_slot(slot), cols)
                elif kind == "dir2":
                    pair3 = flat.rearrange("p (s y) -> p s y", s=2)
                    nc.vector.max(_cand_slot(slot), pair3[:, :, 0:SW])
                elif kind == "cp2":
                    img = m1p.tile([128, 2 * SW], BF16, name="img")
                    pair3 = flat.rearrange("p (s y) -> p s y", s=2)
                    img3 = img.rearrange("p (s w) -> p s w", s=2)
                    nc.scalar.copy(img3[:, :, :], pair3[:, :, 0:SW])
                    pending_folds.append((seq, img, _cand_slot(slot), 2 * SW))
                elif kind == "exp2":
                    img = m1p.tile([128, 2 * SW], BF16, name="img")
                    for wi, s in enumerate(slabs):
                        idx = tile_cams[rt].index(s)
                        cols = cols0 if s % 2 == 0 else cols1
                        dst = img[:, wi * SW : (wi + 1) * SW]
                        _emit_exp(nc, smallp, dst, cols, sc20_sb, rt,
                                  s_ts[rt][:, idx : idx + 1])
                    pending_folds.append((seq, img, _cand_slot(slot), 2 * SW))
                elif kind == "exp1":
                    (s,) = slabs
                    idx = tile_cams[rt].index(s)
                    cols = cols0 if s % 2 == 0 else cols1
                    img = m1p.tile([128, SW], BF16, name="img1")
                    _emit_exp(nc, smallp, img[:, :], cols, sc20_sb, rt,
                              s_ts[rt][:, idx : idx + 1])
                    pending_folds.append((seq, img, _cand_slot(slot), SW))
                elif kind == "cp1":
                    (s,) = slabs
                    cols = cols0 if s % 2 == 0 else cols1
                    img = m1p.tile([128, SW], BF16, name="img1")
                    nc.scalar.copy(img[:, :], cols)
                    pending_folds.append((seq, img, _cand_slot(slot), SW))
                else:  # cp1s: two single copies into one shared image
                    st = cp1s_state.get(id(u))
                    if st is None:
                        st = {"tile": m1p.tile([128, 2 * SW], BF16, name="img"),
                              "done": 0}
                        cp1s_state[id(u)] = st
                    for wi, s in enumerate(slabs):
                        if s // 2 != kpair:
                            continue
                        cols = cols0 if s % 2 == 0 else cols1
                        nc.scalar.copy(
                            st["tile"][:, wi * SW : (wi + 1) * SW], cols
                        )
                        st["done"] += 1
                    if st["done"] == len(slabs):
                        pending_folds.append(
                            (seq, st["tile"], _cand_slot(slot), 2 * SW)
                        )

        # outputs ride the gpsimd SWDGE queues: a cand DMA waiting on folds
        # that execute in the next body must not block the next body's
        # input DMAs (those stay on the sync HWDGE queue)
        nc.gpsimd.dma_start(out=candd[rt], in_=cand_ts[rt][:, :])
        nc.gpsimd.dma_start(out=srowd[rt], in_=s_ts[rt][:, :])


class _Runner:
    """Sharded 8-core executor for a built Bass program (axon/PJRT path)."""

    def __init__(self, nc, n_cores=NCORES):
        import jax
        from jax.sharding import Mesh, PartitionSpec, NamedSharding
        from jax.experimental.shard_map import shard_map
        from concourse import bass2jax

        self.jax = jax
        self.nc = nc
        self.n_cores = n_cores
        bass2jax.install_neuronx_cc_hook()
        partition_name = (
            nc.partition_id_tensor.name if nc.partition_id_tensor else None
        )
        in_names, out_names, out_avals = [], [], []
        for alloc in nc.m.functions[0].allocations:
            if not isinstance(alloc, mybir.MemoryLocationSet):
                continue
            name = alloc.memorylocations[0].name
            if alloc.kind == "ExternalInput":
                if name != partition_name:
                    in_names.append(name)
            elif alloc.kind == "ExternalOutput":
                out_names.append(name)
                out_avals.append(
                    jax.core.ShapedArray(
                        tuple(alloc.tensor_shape), mybir.dt.np(alloc.dtype)
                    )
                )
        self.in_names, self.out_names, self.out_avals = in_names, out_names, out_avals
        n_params, n_outs = len(in_names), len(out_avals)
        all_in_names = list(in_names) + list(out_names)
        if partition_name is not None:
            all_in_names.append(partition_name)

        def _body(*args):
            operands = list(args)
            if partition_name is not None:
                operands.append(bass2jax.partition_id_tensor())
            return tuple(
                bass2jax._bass_exec_p.bind(
                    *operands,
                    out_avals=tuple(out_avals),
                    in_names=tuple(all_in_names),
                    out_names=tuple(out_names),
                    lowering_input_output_aliases=(),
                    sim_require_finite=True,
                    sim_require_nnan=True,
                    nc=nc,
                )
            )

        devices = jax.devices()[:n_cores]
        self.mesh = Mesh(np.asarray(devices), ("core",))
        self.sh = NamedSharding(self.mesh, PartitionSpec("core"))
        self.fn = jax.jit(
            shard_map(
                _body,
                mesh=self.mesh,
                in_specs=(PartitionSpec("core"),) * (n_params + n_outs),
                out_specs=(PartitionSpec("core"),) * n_outs,
                check_rep=False,
            ),
            donate_argnums=tuple(range(n_params, n_params + n_outs)),
            keep_unused=True,
        )
        self._zero_shapes = [
            ((n_cores * a.shape[0], *a.shape[1:]), a.dtype) for a in out_avals
        ]

    def put_inputs(self, in_maps):
        self.dev_in = [
            self.jax.device_put(
                np.concatenate([np.asarray(m[name]) for m in in_maps], axis=0),
                self.sh,
            )
            for name in self.in_names
        ]

    def _zeros(self):
        return [
            self.jax.device_put(np.zeros(s, d), self.sh)
            for s, d in self._zero_shapes
        ]

    def execute(self):
        outs = self.fn(*self.dev_in, *self._zeros())
        self.jax.block_until_ready(outs)
        return self.unpack(outs)

    def unpack(self, outs):
        return [
            {
                name: np.asarray(outs[i]).reshape(
                    self.n_cores, *self.out_avals[i].shape
                )[c]
                for i, name in enumerate(self.out_names)
            }
            for c in range(self.n_cores)
        ]


_RUNNERS = {}
_LAST_FALLBACKS = 0
_FORCE_FALLBACK = False  # test hook: exercise the exact host fallback path


def _get_runner(nc):
    r = _RUNNERS.get(id(nc))
    if r is None:
        r = _Runner(nc)
        _RUNNERS[id(nc)] = r
    return r


def _make_in_maps(cenT_shards, feats_p):
    np_mm = NP_FP8 if MM == "fp8dr" else NP_BF16
    inv = 1.0 / np.linalg.norm(feats_p.astype(np.float64), axis=1)
    sc20 = np.ascontiguousarray(
        (INV_T / CEN_SCALE) * inv.reshape(RT, 128).T, dtype=np.float32
    )  # [128, RT]
    # fT[rt, p, j, m] = feats_p[rt*128 + m, 128*j + p]
    fT = np.ascontiguousarray(
        feats_p.reshape(RT, 128, 2, 128).transpose(0, 3, 2, 1), dtype=np_mm
    )
    return [
        {"cenT": cenT_shards[k], "fT": fT, "sc20": sc20}
        for k in range(NCORES)
    ]


def _host_finish(results, feats_p, labels_p, cams_p, centers, tile_cams, order):
    rows = np.arange(N)
    invn = 1.0 / np.linalg.norm(feats_p.astype(np.float64), axis=1)
    plan = _units(tile_cams)
    inv_order = np.argsort(np.asarray(order))  # camera -> slab

    # chunk tables: (rt, slab) -> covering slot + kind
    slab_slot = np.full((RT, SLABS), -1, dtype=np.int64)
    slab_kind = [[None] * SLABS for _ in range(RT)]
    active = np.zeros((RT, NSLOT), dtype=bool)
    exp_slot = np.zeros((RT, NSLOT), dtype=bool)
    for rt in range(RT):
        for kind, slabs, slot in plan[rt]:
            active[rt, slot] = True
            exp_slot[rt, slot] = kind.startswith("exp")
            for s in slabs:
                slab_slot[rt, s] = slot
                slab_kind[rt][s] = (
                    "exp" if kind.startswith("exp")
                    else ("copy" if kind.startswith("cp") else "direct")
                )

    cand_raw = np.stack(
        [results[k]["cand"].reshape(N, NSLOT, CAND_PER_S) for k in range(NCORES)]
    ).astype(np.float64)  # [8, 512, NSLOT, 8]
    cscale = invn / CEN_SCALE
    rt_of = rows // 128
    is_exp = exp_slot[rt_of]                       # [512, NSLOT]
    act = active[rt_of]                            # [512, NSLOT]
    cand = np.where(
        is_exp[None, :, :, None],
        np.log(np.maximum(cand_raw, 1e-30)) / INV_T,
        cand_raw * cscale[None, :, None, None],
    )
    cand = np.where(act[None, :, :, None], cand, -np.inf)

    # srow slots: per row-tile, slot idx corresponds to tile_cams order
    # (tile_cams holds SLAB indices; row's slab = inv_order[cam])
    slab_of_row = inv_order[cams_p]
    slot = np.zeros(N, dtype=np.int64)
    for rt in range(RT):
        for idx, sl in enumerate(tile_cams[rt]):
            sel = slice(128 * rt, 128 * (rt + 1))
            slot[sel] = np.where(slab_of_row[sel] == sl, idx, slot[sel])
    p_of = rows % 128
    s_k = np.stack(
        [
            results[k]["srow"].reshape(RT, 128, C)[rt_of, p_of, slot]
            for k in range(NCORES)
        ]
    ).astype(np.float64)  # [8, 512]

    fe = feats_p.astype(np.float64)
    fn = fe / np.linalg.norm(fe, axis=1, keepdims=True)
    cen = centers.astype(np.float64)

    # positives: 8 same-label proxies per row (host, f64)
    gidx = labels_p[:, None] * C + np.arange(C)[None, :]        # [512, 8]
    pos = np.einsum("rcd,rd->rc", cen[gidx], fn)                # [512, 8]

    # ---- intra ----
    lse_intra = np.log(s_k.sum(axis=0))
    v = pos[np.arange(N), cams_p]
    loss_intra_i = lse_intra - INV_T * v

    # ---- inter: remove positives from candidates by value, then top-50 ----
    np_mm = NP_FP8 if MM == "fp8dr" else NP_BF16
    f_q = feats_p.astype(np_mm).astype(np.float64)
    g_q = (CEN_SCALE * centers[gidx]).astype(np_mm).astype(np.float64)
    pos_dev = np.einsum("rcd,rd->rc", g_q, f_q).astype(np.float32)  # raw dot
    sc20r = (INV_T / CEN_SCALE) * invn
    pred_exp = (
        np.log(
            np.exp(sc20r[:, None] * pos_dev.astype(np.float64))
            .astype(NP_BF16).astype(np.float64)
        ) / INV_T
    )
    pred_raw_b = pos_dev.astype(NP_BF16).astype(np.float64) * cscale[:, None]
    pred_raw_x = pos_dev.astype(np.float64) * cscale[:, None]

    CRS = cand.transpose(1, 0, 2, 3)                # [512, 8cores, NSLOT, 8]
    owner = labels_p // L_LOCAL
    for i in rows:
        rt = i // 128
        for c in range(C):
            s = inv_order[c]
            kind = slab_kind[rt][s]
            sl = slab_slot[rt, s]
            if kind == "exp":
                pv = pred_exp[i, c]
            elif kind == "copy":
                pv = pred_raw_b[i, c]
            else:
                pv = pred_raw_x[i, c]
            vals = CRS[i, owner[i], sl]
            d = np.abs(vals - pv)
            j = np.argmin(d)
            if d[j] < 2.5e-4 + 5e-3 * abs(pv):
                CRS[i, owner[i], sl, j] = -np.inf

    CR = CRS.reshape(N, NCORES * NSLOT * CAND_PER_S)
    nc_tot = CR.shape[1]
    part = np.partition(CR, nc_tot - K, axis=1)[:, -K:]  # top-50 values
    t50 = part.min(axis=1)

    # at-risk check: each chunk's 8th-largest candidate should be <= t50
    # (sound certificate for 'direct' chunks, heuristic for folded chunks)
    slab8 = np.where(act[None], cand[:, :, :, CAND_PER_S - 1], -np.inf)
    if _FORCE_FALLBACK:
        bad = rows
    else:
        bad = np.where(slab8.max(axis=(0, 2)) > t50)[0]
    global _LAST_FALLBACKS
    _LAST_FALLBACKS = len(bad)
    if len(bad):
        sims_bad = fn[bad] @ cen.T                              # [nbad, 64000]
        for bi, i in enumerate(bad):
            srow = sims_bad[bi]
            srow[C * labels_p[i] : C * labels_p[i] + C] = -np.inf
            part[i] = np.sort(srow)[-K:]

    z = np.concatenate([pos, part], axis=1) * INV_T             # [512, 58]
    mz = z.max(axis=1)
    lse_inter = np.log(np.exp(z - mz[:, None]).sum(axis=1)) + mz
    loss_inter_i = lse_inter - INV_T * pos.mean(axis=1)

    # ---- per-camera means, summed ----
    cnt = np.bincount(cams_p, minlength=C).astype(np.float64)
    s_intra = np.bincount(cams_p, weights=loss_intra_i, minlength=C)
    s_inter = np.bincount(cams_p, weights=loss_inter_i, minlength=C)
    safe = np.maximum(cnt, 1.0)
    li = np.sum(np.where(cnt > 0, s_intra / safe, 0.0))
    le = LW * np.sum(np.where(cnt > 0, s_inter / safe, 0.0))
    return np.array([li, le], dtype=np.float32)


def _prepare(feats, indexes, label_table, cam_table, centers):
    feats = np.asarray(feats, dtype=np.float32)
    indexes = np.asarray(indexes)
    label_table = np.asarray(label_table)
    cam_table = np.asarray(cam_table)
    centers = np.asarray(centers, dtype=np.float32)

    labels = np.asarray(label_table[indexes], dtype=np.int64)
    cams = np.asarray(cam_table[indexes], dtype=np.int64)

    # permute rows so camera groups are contiguous, ordered so most 128-row
    # tiles span only ~2 cameras (fewer intra exp instructions)
    sizes = np.bincount(cams, minlength=C)
    order = _pair_order(sizes)
    perm = np.concatenate([np.where(cams == c)[0] for c in order])
    feats_p = np.ascontiguousarray(feats[perm])
    labels_p = labels[perm]
    cams_p = cams[perm]
    inv_order = np.argsort(np.asarray(order))  # camera -> slab index
    slabs_p = inv_order[cams_p]
    # tile_cams in SLAB indices: consecutive runs by construction
    tile_cams = tuple(
        tuple(dict.fromkeys(slabs_p[128 * rt : 128 * (rt + 1)].tolist()))
        for rt in range(RT)
    )

    # per-core centers, slab s = camera order[s], 24-col pad per slab,
    # pre-scaled, transposed to [128, 2, PL] (partition=feat_lo, j=feat_hi)
    np_mm = NP_FP8 if MM == "fp8dr" else NP_BF16
    by_cam = centers.reshape(L, C, D)
    cenT_shards = []
    for k in range(NCORES):
        X = by_cam[k * L_LOCAL : (k + 1) * L_LOCAL]             # [1000, 8, 256]
        CP = np.zeros((C, SLABW, D), dtype=np.float32)
        for s in range(C):
            CP[s, 0:SW] = X[:, order[s], :]
        CP = (CEN_SCALE * CP).reshape(PL, 2, 128)
        cenT_shards.append(
            np.ascontiguousarray(CP.transpose(2, 1, 0), dtype=np_mm)
        )
    return centers, tile_cams, feats_p, labels_p, cams_p, cenT_shards, order


def kernel(feats, indexes, label_table, cam_table, centers):
    centers, tile_cams, feats_p, labels_p, cams_p, cenT_shards, order = _prepare(
        feats, indexes, label_table, cam_table, centers
    )
    nc = _build_program(tile_cams)
    runner = _get_runner(nc)
    runner.put_inputs(_make_in_maps(cenT_shards, feats_p))
    results = runner.execute()
    return _host_finish(
        results, feats_p, labels_p, cams_p, centers, tile_cams, order
    )


# revision 37
# speedup vs baseline: 1.3423x; 1.3423x over previous
"""Trainium2 Bass kernel for nn_CAPMemory (camera-aware proxy memory loss).

Strategy (8 NeuronCores, SPMD, no collectives):
  - Shard the 64000x256 proxy table over P: core k owns labels
    [1000k, 1000(k+1)), all 8 cameras. Per-core column layout is CAM-MAJOR
    in the ROW-PERMUTATION camera order (slab s holds camera order[s]), one
    1024-col slab per camera (1000 real + 24 zero-pad). Slabs are grouped in
    PAIRS sharing one 4-bank PSUM tile, so a single drain instruction can
    read 2000 real columns with one fixed-overhead charge. Because rows are
    permuted so camera groups are contiguous in the same order, each row
    tile's exp cameras form a consecutive slab run, leaving the non-exp
    slabs as aligned pairs.
  - Matmuls run in fp8(e4m3) DoubleRow mode: operands laid out [128, 2, free]
    so one matmul contracts all K=256 at 2 MACs/cell/cycle. Centers are
    pre-scaled by 16 on the host so their entries (~N(0,1/256)) sit in e4m3's
    normal range; the 1/16 rides in the host post-scale and the exp scale.
    feats are transposed/quantized on the host (fT input); row norms arrive
    as the sc20 input. Centers SBUF is double-buffered so the fp8 DMA of the
    next iteration hides under compute.
  - PSUM drain per pair-tile, unit kinds chosen statically to balance ACT
    and DVE busy time (exact cost-model constants):
      exp  : per-slab ACT exp (scale=sc20) -> bf16 image half + accum_out
             (intra denominator). Monotone, so the image top-8 are the
             chunk candidates (exp domain). Not pairable (accum per slab).
      cp2  : ONE ACT copy over both slabs [128,2,1000] -> 2000-wide bf16
             image (raw domain).
      dir2 : ONE DVE InstMax over both slabs (exact top-8 of the 2000).
      cp1/dir1: single-slab fallbacks for orphan slabs next to an exp slab.
    Images are folded on DVE by pairwise tensor_max (2x bf16) down to <=256
    stripe maxima and finished with one InstMax into the unit's candidate
    slot.
  - Candidates: top-8 per unit chunk; host merges 8 cores' candidates,
    removes positives by value-matching, takes top-50, and recomputes rows
    whose per-chunk 8th-largest exceeds the merged t50 exactly (fallback).
  - Host merge: intra logsumexp = log(sum_k srow_k); positives in f64;
    per-camera means as in the reference.
"""

import os
import sys
import functools

sys.path.insert(0, "/opt/trn_rl_repo")

import numpy as np

from concourse import bacc, mybir
from concourse.tile import TileContext

F32 = mybir.dt.float32
BF16 = mybir.dt.bfloat16
FP8 = mybir.dt.float8e4
NP_FP8 = mybir.dt.np(FP8)
NP_BF16 = mybir.dt.np(BF16)

N = 512          # batch
D = 256          # feature dim
L = 8000         # labels
C = 8            # cameras
NCORES = 8
L_LOCAL = 1000   # labels per core
RT = 4           # row tiles of 128
SLABW = 1024     # padded columns per camera slab (1000 + 24 pad)
PL = 8 * SLABW   # padded per-core columns (8192)
SW = 1000        # real slab width (one camera's columns)
INV_T = 20.0     # 1 / temperature
K = 50           # hard negatives
LW = 0.5         # inter-cam loss weight
CEN_SCALE = 16.0 # host pre-scale on centers (keeps fp8 in normal range)
CAND_PER_S = 8
SLABS = C
NSLOT = 8                     # candidate slots per (rt, core)
CAND = NSLOT * CAND_PER_S     # 64 candidate values per row-tile per core

# experiment knobs
MM = os.environ.get("V2_MM", "fp8dr")            # fp8dr|bf16
FOLDS_TGT = int(os.environ.get("V2_FOLDS_TGT", "256"))  # fold down to <= this
M1BUFS = int(os.environ.get("V2_M1BUFS", "6"))   # scr/fold tile ring depth
DEFER = int(os.environ.get("V2_DEFER", "2"))     # units to defer fold chains by
CP2_OVR = os.environ.get("V3_CP2", "4")          # dir2 pair count override
PLAN_MODE = os.environ.get("V3_MODE", "dir2")    # dir2|mix2
POOLSUM = os.environ.get("V3_POOLSUM", "0") == "1"  # intra sums on gpsimd

# cost-model constants (ns) used by the static ACT/DVE balance
_ACT_EXP = 1205.0    # single-slab exp with accum read
_ACT_EXP_NOACC = 1018.0  # exp without accum (POOLSUM mode)
_ACT_CP2 = 1852.0    # paired copy [128,2,1000]
_ACT_CP1 = 1018.0    # single copy
_DVE_DIR2 = 2208.0   # paired InstMax from PSUM
_DVE_DIR1 = 1167.0   # single InstMax from PSUM
_DVE_F2K = 1414.0    # fold chain + InstMax for a 2000-wide bf16 image
_DVE_F1K = 833.0     # fold chain + InstMax for a 1000-wide bf16 image
_ACT_BUBBLE = 773.0  # ACT refill bubble per pure-DVE pair drain


def _pair_order(sizes):
    """Order cameras so as few camera blocks as possible cross a 128-row
    tile boundary: every crossing costs one extra intra-exp instruction.
    8! is tiny, so search exhaustively."""
    from itertools import permutations

    sizes = [int(s) for s in sizes]
    best, best_cross = None, None
    for perm in permutations(range(C)):
        acc = cross = 0
        for c in perm:
            lo = acc
            acc += sizes[c]
            cross += (acc - 1) // 128 - lo // 128 if sizes[c] else 0
        if best_cross is None or cross < best_cross:
            best, best_cross = perm, cross
            if cross == 0:
                break
    return list(best)


def _units(tile_cams):
    """Static drain plan shared by device build and host decode.

    Slab s holds camera order[s]; tile_cams is given in SLAB indices here
    (i.e. already mapped through the order). Returns units[rt] = list of
      (kind, slabs, slot) with kind in {exp2, exp1, cp2, cp1s, cp1, dir1}
    where cp1s is a pair of single copies sharing one 2000 image (slabs may
    be non-adjacent). slot is the candidate slot index.

    Pure-DVE pair drains (dir2) are avoided: with the 2-deep PSUM ring a
    pair with no ACT work exposes the next refill (~770ns) as an ACT
    bubble. Instead DVE PSUM work comes from 'mix2' pairs = one slab
    dir1 (DVE) + one slab cp1 (ACT), so every pair keeps ACT busy while
    its partner tile refills.
    """
    free_pairs = []   # (rt, k)
    orphans = []      # (rt, s) slabs next to an exp1 in their pair
    n_exp_i = 0
    n_f2k = 0         # exp2 fold images
    n_f1k = 0         # exp1 fold images
    for rt in range(RT):
        E = set(tile_cams[rt])
        for k in range(SLABS // 2):
            a, b = 2 * k, 2 * k + 1
            ina, inb = a in E, b in E
            if ina and inb:
                n_exp_i += 2
                n_f2k += 1
            elif ina or inb:
                e, o = (a, b) if ina else (b, a)
                n_exp_i += 1
                n_f1k += 1
                orphans.append((rt, o))
            else:
                free_pairs.append((rt, k))

    n_dir1_orph = len(orphans)  # orphans default to dir1
    exp_cost = _ACT_EXP_NOACC if POOLSUM else _ACT_EXP

    P = len(free_pairs)
    by_rt_free = {}
    for rt, k in free_pairs:
        by_rt_free.setdefault(rt, []).append(k)

    def _distribute(m, role):
        """Round-robin `role` across rts (latest free pair of each rt
        first); returns {(rt,k): role|'cp2'} and per-rt count."""
        roles = {fp: "cp2" for fp in free_pairs}
        cnt = {rt: 0 for rt in range(RT)}
        avail = {rt: list(reversed(ks)) for rt, ks in by_rt_free.items()}
        left = m
        for rt in (1, 2, 3, 0) * 4:
            if left <= 0:
                break
            ks = avail.get(rt)
            if ks:
                roles[(rt, ks.pop(0))] = role
                cnt[rt] += 1
                left -= 1
        return roles, cnt

    best_m, best_t = 0, None
    for m in range(P + 1):
        _, cnt = _distribute(m, "x")
        if PLAN_MODE == "mix2":
            n_sh = sum(c // 2 for c in cnt.values())       # shared cp1s
            n_single = sum(c % 2 for c in cnt.values())    # standalone cp1
            act = n_exp_i * exp_cost + (P - m) * _ACT_CP2 + m * _ACT_CP1
            dve = (
                (n_f2k + (P - m) + n_sh) * _DVE_F2K
                + (n_f1k + n_single) * _DVE_F1K
                + (m + n_dir1_orph) * _DVE_DIR1
            )
        else:  # dir2: m pure-DVE pair drains; each exposes an ACT bubble
            act = (
                n_exp_i * exp_cost + (P - m) * _ACT_CP2 + m * _ACT_BUBBLE
            )
            dve = (
                (n_f2k + (P - m)) * _DVE_F2K
                + n_f1k * _DVE_F1K
                + m * _DVE_DIR2
                + n_dir1_orph * _DVE_DIR1
            )
        t = max(act, dve)
        if best_t is None or t < best_t:
            best_t, best_m = t, m
    m = int(CP2_OVR) if CP2_OVR else best_m
    role_name = "mix2" if PLAN_MODE == "mix2" else "dir2"
    roles, _ = _distribute(m, role_name)

    out = []
    for rt in range(RT):
        units = []
        E = set(tile_cams[rt])
        cp1_halves = []
        for k in range(SLABS // 2):
            a, b = 2 * k, 2 * k + 1
            ina, inb = a in E, b in E
            if ina and inb:
                units.append(["exp2", (a, b)])
            elif ina or inb:
                e, o = (a, b) if ina else (b, a)
                units.append(["exp1", (e,)])
                units.append(["dir1", (o,)])
            elif roles[(rt, k)] == "cp2":
                units.append(["cp2", (a, b)])
            elif roles[(rt, k)] == "dir2":
                units.append(["dir2", (a, b)])
            else:  # mix2: even slab -> dir1 (DVE), odd slab -> cp1 (ACT)
                units.append(["dir1", (a,)])
                cp1_halves.append(b)
        i = 0
        while i + 1 < len(cp1_halves):
            units.append(["cp1s", (cp1_halves[i], cp1_halves[i + 1])])
            i += 2
        if i < len(cp1_halves):
            units.append(["cp1", (cp1_halves[i],)])
        final = []
        for slot, (kind, slabs) in enumerate(units):
            final.append((kind, tuple(slabs), slot))
        assert len(final) <= NSLOT, final
        out.append(final)
    return out


@functools.lru_cache(maxsize=8)
def _build_program(tile_cams, repeats=1):
    nc = bacc.Bacc(None, target_bir_lowering=False, num_swdge_queues=4)

    mm_dt = FP8 if MM == "fp8dr" else BF16
    cenT = nc.dram_tensor("cenT", [128, 2, PL], mm_dt, kind="ExternalInput")
    fTd = nc.dram_tensor("fT", [RT, 128, 2, 128], mm_dt, kind="ExternalInput")
    sc20d = nc.dram_tensor("sc20", [128, RT], F32, kind="ExternalInput")
    candd = nc.dram_tensor("cand", [RT, 128, CAND], F32, kind="ExternalOutput")
    srowd = nc.dram_tensor("srow", [RT, 128, C], F32, kind="ExternalOutput")

    with TileContext(nc) as tc:
        with (
            tc.tile_pool(name="cen", bufs=2) as cenp,
            tc.tile_pool(name="ftp", bufs=2) as ftp,
            tc.tile_pool(name="m1p", bufs=M1BUFS) as m1p,
            tc.tile_pool(name="smallp", bufs=2) as smallp,
            tc.tile_pool(name="outp", bufs=2) as outp,
            tc.tile_pool(name="psum", bufs=2, space="PSUM") as psump,
        ):
            # fold chains carry across iteration bodies (flushed with a
            # fixed defer in global pair order); final flush after the
            # last body
            state = {"pending": [], "base": 0}
            for _rep in range(repeats):
                _kernel_body(nc, tc, cenp, ftp, m1p, smallp, outp, psump,
                             cenT, fTd, sc20d, candd, srowd, tile_cams,
                             state, warm=(_rep == 0))
                state["base"] += RT * (SLABS // 2)
            for ent in state["pending"]:
                _fold_and_max(nc, m1p, ent[2], ent[1], ent[3])
                if ent[4] is not None:
                    ent[4]()

    nc.compile()
    return nc


def _emit_exp(nc, smallp, dst, cols, sc20_sb, rt, accum_ap):
    """ACT exp into a bf16 image half; intra sum via ACT accum_out, or (in
    POOLSUM mode) via a gpsimd tensor_scalar pass over the image."""
    ActF = mybir.ActivationFunctionType
    if not POOLSUM:
        nc.scalar.activation(
            dst, cols, ActF.Exp,
            scale=sc20_sb[:, rt : rt + 1],
            accum_out=accum_ap,
        )
        return
    nc.scalar.activation(dst, cols, ActF.Exp,
                         scale=sc20_sb[:, rt : rt + 1])
    dummy = smallp.tile([128, SW], BF16, name="psdummy", bufs=2)
    nc.gpsimd.tensor_scalar(
        dummy[:, :], dst, 1.0, None, mybir.AluOpType.mult,
        accum_out=accum_ap,
    )


def _fold_and_max(nc, m1p, co, img, w):
    """DVE: pairwise tensor_max folds (2x bf16 mode) down to <=FOLDS_TGT
    stripe maxima, then InstMax top-8."""
    cur = img
    while w > FOLDS_TGT and w % 2 == 0:
        half = w // 2
        nxt = m1p.tile([128, half], BF16, name="fold")
        nc.vector.tensor_max(nxt[:, :], cur[:, 0:half], cur[:, half : 2 * half])
        cur, w = nxt, half
    nc.vector.max(co, cur[:, 0:w])


def _kernel_body(nc, tc, cenp, ftp, m1p, smallp, outp, psump,
                 cenT, fTd, sc20d, candd, srowd, tile_cams, state,
                 warm=True):
    ActF = mybir.ActivationFunctionType
    mm_dt = FP8 if MM == "fp8dr" else BF16
    plan = _units(tile_cams)

    # small transfers first; warm the Exp LUT in ACT's idle window
    # (first body only — the table stays resident across repeats)
    sc20_sb = smallp.tile([128, RT], F32, name="sc20", bufs=2)
    nc.sync.dma_start(out=sc20_sb[:, :], in_=sc20d[:, :])
    if warm:
        warm_t = smallp.tile([128, 1], F32, name="warm", bufs=2)
        nc.scalar.activation(warm_t[:, 0:1], sc20_sb[:, 0:1], ActF.Exp)

    fTs = []
    for rt in range(RT):
        fT = ftp.tile([128, 2, 128], mm_dt, name=f"fT{rt}")
        nc.sync.dma_start(out=fT[:, :, :], in_=fTd[rt])
        fTs.append(fT)

    # centers: one DMA per 2-slab group (a matmul then waits on a single
    # completion sem); alternate queues so transfers overlap
    # inputs on the sync/vector HWDGE queues (outputs use gpsimd SWDGE;
    # inputs must not share a queue with outputs or a waiting output
    # blocks them). One DMA per 2-slab group: a matmul waits one sem.
    cen_sb = cenp.tile([128, 2, PL], mm_dt, name="cen")
    for g in range(4):
        s = slice(g * 2 * SLABW, (g + 1) * 2 * SLABW)
        nc.sync.dma_start(out=cen_sb[:, :, s], in_=cenT[:, :, s])

    cand_ts = [
        outp.tile([128, CAND], F32, name=f"cand{rt}", bufs=2) for rt in range(RT)
    ]
    s_ts = [
        smallp.tile([128, C], F32, name=f"s_t{rt}", bufs=2) for rt in range(RT)
    ]

    # pending entries: [seq, img_tile, cand_slot_ap, width, post_cb]
    # post_cb fires after the fold is emitted — used to emit an rt's cand
    # DMA only once every fold writing that cand tile has been emitted
    # (emitting the DMA earlier would miss those writers in its deps).
    pending_folds = state["pending"]

    def _flush_folds(upto):
        while pending_folds and pending_folds[0][0] <= upto - DEFER:
            ent = pending_folds.pop(0)
            _fold_and_max(nc, m1p, ent[2], ent[1], ent[3])
            if ent[4] is not None:
                ent[4]()

    for rt in range(RT):
        units = plan[rt]

        def _cand_slot(slot):
            return cand_ts[rt][:, slot * CAND_PER_S : (slot + 1) * CAND_PER_S]

        # units indexed by slab; pair-wide units (exp2/cp2) fire once
        unit_of = {}
        for u in units:
            kind, slabs, slot = u
            for s in slabs:
                unit_of[s] = u

        # shared-image state for cp1s units, keyed by unit id
        cp1s_state = {}

        for kpair in range(SLABS // 2):
            seq = state["base"] + rt * (SLABS // 2) + kpair
            _flush_folds(seq)
            ps = psump.tile([128, 4, 512], F32, name="ps")
            for sub in range(2):
                s = 2 * kpair + sub
                for mk in range(2):
                    lo = s * SLABW + mk * 512
                    if MM == "fp8dr":
                        nc.tensor.matmul(
                            ps[:, 2 * sub + mk, :], fTs[rt][:, :, :],
                            cen_sb[:, :, lo : lo + 512],
                            start=True, stop=True,
                            perf_mode=mybir.MatmulPerfMode.DoubleRow,
                        )
                    else:
                        nc.tensor.matmul(
                            ps[:, 2 * sub + mk, :], fTs[rt][:, 0, :],
                            cen_sb[:, 0, lo : lo + 512],
                            start=True, stop=False,
                        )
                        nc.tensor.matmul(
                            ps[:, 2 * sub + mk, :], fTs[rt][:, 1, :],
                            cen_sb[:, 1, lo : lo + 512],
                            start=False, stop=True,
                        )

            flat = ps.rearrange("p a b -> p (a b)")     # [128, 2048]
            cols0 = flat[:, 0:SW]                        # slab 2k real cols
            cols1 = flat[:, SLABW : SLABW + SW]          # slab 2k+1 real cols

            # pair-wide units fire once; per-slab units fire per slab.
            # DVE dir1 drains are emitted FIRST so they sit ahead of this
            # pair's fold work in the DVE queue (they release PSUM).
            pair_us = []
            seen = set()
            for s in (2 * kpair, 2 * kpair + 1):
                u = unit_of[s]
                if id(u) not in seen:
                    seen.add(id(u))
                    pair_us.append(u)
            pair_us.sort(key=lambda u: u[0] != "dir1")

            for u in pair_us:
                kind, slabs, slot = u
                if kind == "dir1":
                    for s in slabs:
                        if s // 2 == kpair:
                            cols = cols0 if s % 2 == 0 else cols1
                            nc.vector.max(_cand_slot(slot), cols)
                elif kind == "dir2":
                    pair3 = flat.rearrange("p (s y) -> p s y", s=2)
                    nc.vector.max(_cand_slot(slot), pair3[:, :, 0:SW])
                elif kind == "cp2":
                    img = m1p.tile([128, 2 * SW], BF16, name="img")
                    pair3 = flat.rearrange("p (s y) -> p s y", s=2)
                    img3 = img.rearrange("p (s w) -> p s w", s=2)
                    nc.scalar.copy(img3[:, :, :], pair3[:, :, 0:SW])
                    pending_folds.append([seq, img, _cand_slot(slot), 2 * SW, None, rt])
                elif kind == "exp2":
                    img = m1p.tile([128, 2 * SW], BF16, name="img")
                    for wi, s in enumerate(slabs):
                        idx = tile_cams[rt].index(s)
                        cols = cols0 if s % 2 == 0 else cols1
                        dst = img[:, wi * SW : (wi + 1) * SW]
                        _emit_exp(nc, smallp, dst, cols, sc20_sb, rt,
                                  s_ts[rt][:, idx : idx + 1])
                    pending_folds.append([seq, img, _cand_slot(slot), 2 * SW, None, rt])
                elif kind == "exp1":
                    (s,) = slabs
                    idx = tile_cams[rt].index(s)
                    cols = cols0 if s % 2 == 0 else cols1
                    img = m1p.tile([128, SW], BF16, name="img1")
                    _emit_exp(nc, smallp, img[:, :], cols, sc20_sb, rt,
                              s_ts[rt][:, idx : idx + 1])
                    pending_folds.append([seq, img, _cand_slot(slot), SW, None, rt])
                elif kind == "cp1":
                    (s,) = slabs
                    cols = cols0 if s % 2 == 0 else cols1
                    img = m1p.tile([128, SW], BF16, name="img1")
                    nc.scalar.copy(img[:, :], cols)
                    pending_folds.append([seq, img, _cand_slot(slot), SW, None, rt])
                else:  # cp1s: two single copies into one shared image
                    st = cp1s_state.get(id(u))
                    if st is None:
                        st = {"tile": m1p.tile([128, 2 * SW], BF16, name="img"),
                              "done": 0}
                        cp1s_state[id(u)] = st
                    for wi, s in enumerate(slabs):
                        if s // 2 != kpair:
                            continue
                        cols = cols0 if s % 2 == 0 else cols1
                        nc.scalar.copy(
                            st["tile"][:, wi * SW : (wi + 1) * SW], cols
                        )
                        st["done"] += 1
                    if st["done"] == len(slabs):
                        pending_folds.append(
                            [seq, st["tile"], _cand_slot(slot), 2 * SW, None, rt]
                        )

        # outputs ride the gpsimd SWDGE queues: a cand DMA waiting on folds
        # that execute in the next body must not block the next body's
        # input DMAs (those stay on the sync HWDGE queue). The cand DMA
        # must be EMITTED after the rt's last fold so the fold is in its
        # dependency set; srow's writers (exps) are all emitted by now.
        def _emit_cand_dma(rt=rt, tile=cand_ts[rt]):
            nc.gpsimd.dma_start(out=candd[rt], in_=tile[:, :])
        mine = [e for e in pending_folds if e[5] == rt]
        if mine:
            prev_cb = mine[-1][4]
            mine[-1][4] = (
                _emit_cand_dma if prev_cb is None
                else (lambda p=prev_cb: (p(), _emit_cand_dma()))
            )
        else:
            _emit_cand_dma()
        nc.gpsimd.dma_start(out=srowd[rt], in_=s_ts[rt][:, :])


class _Runner:
    """Sharded 8-core executor for a built Bass program (axon/PJRT path)."""

    def __init__(self, nc, n_cores=NCORES):
        import jax
        from jax.sharding import Mesh, PartitionSpec, NamedSharding
        from jax.experimental.shard_map import shard_map
        from concourse import bass2jax

        self.jax = jax
        self.nc = nc
        self.n_cores = n_cores
        bass2jax.install_neuronx_cc_hook()
        partition_name = (
            nc.partition_id_tensor.name if nc.partition_id_tensor else None
        )
        in_names, out_names, out_avals = [], [], []
        for alloc in nc.m.functions[0].allocations:
            if not isinstance(alloc, mybir.MemoryLocationSet):
                continue
            name = alloc.memorylocations[0].name
            if alloc.kind == "ExternalInput":
                if name != partition_name:
                    in_names.append(name)
            elif alloc.kind == "ExternalOutput":
                out_names.append(name)
                out_avals.append(
                    jax.core.ShapedArray(
                        tuple(alloc.tensor_shape), mybir.dt.np(alloc.dtype)
                    )
                )
        self.in_names, self.out_names, self.out_avals = in_names, out_names, out_avals
        n_params, n_outs = len(in_names), len(out_avals)
        all_in_names = list(in_names) + list(out_names)
        if partition_name is not None:
            all_in_names.append(partition_name)

        def _body(*args):
            operands = list(args)
            if partition_name is not None:
                operands.append(bass2jax.partition_id_tensor())
            return tuple(
                bass2jax._bass_exec_p.bind(
                    *operands,
                    out_avals=tuple(out_avals),
                    in_names=tuple(all_in_names),
                    out_names=tuple(out_names),
                    lowering_input_output_aliases=(),
                    sim_require_finite=True,
                    sim_require_nnan=True,
                    nc=nc,
                )
            )

        devices = jax.devices()[:n_cores]
        self.mesh = Mesh(np.asarray(devices), ("core",))
        self.sh = NamedSharding(self.mesh, PartitionSpec("core"))
        self.fn = jax.jit(
            shard_map(
                _body,
                mesh=self.mesh,
                in_specs=(PartitionSpec("core"),) * (n_params + n_outs),
                out_specs=(PartitionSpec("core"),) * n_outs,
                check_rep=False,
            ),
            donate_argnums=tuple(range(n_params, n_params + n_outs)),
            keep_unused=True,
        )
        self._zero_shapes = [
            ((n_cores * a.shape[0], *a.shape[1:]), a.dtype) for a in out_avals
        ]

    def put_inputs(self, in_maps):
        self.dev_in = [
            self.jax.device_put(
                np.concatenate([np.asarray(m[name]) for m in in_maps], axis=0),
                self.sh,
            )
            for name in self.in_names
        ]

    def _zeros(self):
        return [
            self.jax.device_put(np.zeros(s, d), self.sh)
            for s, d in self._zero_shapes
        ]

    def execute(self):
        outs = self.fn(*self.dev_in, *self._zeros())
        self.jax.block_until_ready(outs)
        return self.unpack(outs)

    def unpack(self, outs):
        return [
            {
                name: np.asarray(outs[i]).reshape(
                    self.n_cores, *self.out_avals[i].shape
                )[c]
                for i, name in enumerate(self.out_names)
            }
            for c in range(self.n_cores)
        ]


_RUNNERS = {}
_LAST_FALLBACKS = 0
_FORCE_FALLBACK = False  # test hook: exercise the exact host fallback path


def _get_runner(nc):
    r = _RUNNERS.get(id(nc))
    if r is None:
        r = _Runner(nc)
        _RUNNERS[id(nc)] = r
    return r


def _make_in_maps(cenT_shards, feats_p):
    np_mm = NP_FP8 if MM == "fp8dr" else NP_BF16
    inv = 1.0 / np.linalg.norm(feats_p.astype(np.float64), axis=1)
    sc20 = np.ascontiguousarray(
        (INV_T / CEN_SCALE) * inv.reshape(RT, 128).T, dtype=np.float32
    )  # [128, RT]
    # fT[rt, p, j, m] = feats_p[rt*128 + m, 128*j + p]
    fT = np.ascontiguousarray(
        feats_p.reshape(RT, 128, 2, 128).transpose(0, 3, 2, 1), dtype=np_mm
    )
    return [
        {"cenT": cenT_shards[k], "fT": fT, "sc20": sc20}
        for k in range(NCORES)
    ]


def _host_finish(results, feats_p, labels_p, cams_p, centers, tile_cams, order):
    rows = np.arange(N)
    invn = 1.0 / np.linalg.norm(feats_p.astype(np.float64), axis=1)
    plan = _units(tile_cams)
    inv_order = np.argsort(np.asarray(order))  # camera -> slab

    # chunk tables: (rt, slab) -> covering slot + kind
    slab_slot = np.full((RT, SLABS), -1, dtype=np.int64)
    slab_kind = [[None] * SLABS for _ in range(RT)]
    active = np.zeros((RT, NSLOT), dtype=bool)
    exp_slot = np.zeros((RT, NSLOT), dtype=bool)
    for rt in range(RT):
        for kind, slabs, slot in plan[rt]:
            active[rt, slot] = True
            exp_slot[rt, slot] = kind.startswith("exp")
            for s in slabs:
                slab_slot[rt, s] = slot
                slab_kind[rt][s] = (
                    "exp" if kind.startswith("exp")
                    else ("copy" if kind.startswith("cp") else "direct")
                )

    cand_raw = np.stack(
        [results[k]["cand"].reshape(N, NSLOT, CAND_PER_S) for k in range(NCORES)]
    ).astype(np.float64)  # [8, 512, NSLOT, 8]
    cscale = invn / CEN_SCALE
    rt_of = rows // 128
    is_exp = exp_slot[rt_of]                       # [512, NSLOT]
    act = active[rt_of]                            # [512, NSLOT]
    cand = np.where(
        is_exp[None, :, :, None],
        np.log(np.maximum(cand_raw, 1e-30)) / INV_T,
        cand_raw * cscale[None, :, None, None],
    )
    cand = np.where(act[None, :, :, None], cand, -np.inf)

    # srow slots: per row-tile, slot idx corresponds to tile_cams order
    # (tile_cams holds SLAB indices; row's slab = inv_order[cam])
    slab_of_row = inv_order[cams_p]
    slot = np.zeros(N, dtype=np.int64)
    for rt in range(RT):
        for idx, sl in enumerate(tile_cams[rt]):
            sel = slice(128 * rt, 128 * (rt + 1))
            slot[sel] = np.where(slab_of_row[sel] == sl, idx, slot[sel])
    p_of = rows % 128
    s_k = np.stack(
        [
            results[k]["srow"].reshape(RT, 128, C)[rt_of, p_of, slot]
            for k in range(NCORES)
        ]
    ).astype(np.float64)  # [8, 512]

    fe = feats_p.astype(np.float64)
    fn = fe / np.linalg.norm(fe, axis=1, keepdims=True)
    cen = centers.astype(np.float64)

    # positives: 8 same-label proxies per row (host, f64)
    gidx = labels_p[:, None] * C + np.arange(C)[None, :]        # [512, 8]
    pos = np.einsum("rcd,rd->rc", cen[gidx], fn)                # [512, 8]

    # ---- intra ----
    lse_intra = np.log(s_k.sum(axis=0))
    v = pos[np.arange(N), cams_p]
    loss_intra_i = lse_intra - INV_T * v

    # ---- inter: remove positives from candidates by value, then top-50 ----
    np_mm = NP_FP8 if MM == "fp8dr" else NP_BF16
    f_q = feats_p.astype(np_mm).astype(np.float64)
    g_q = (CEN_SCALE * centers[gidx]).astype(np_mm).astype(np.float64)
    pos_dev = np.einsum("rcd,rd->rc", g_q, f_q).astype(np.float32)  # raw dot
    sc20r = (INV_T / CEN_SCALE) * invn
    pred_exp = (
        np.log(
            np.exp(sc20r[:, None] * pos_dev.astype(np.float64))
            .astype(NP_BF16).astype(np.float64)
        ) / INV_T
    )
    pred_raw_b = pos_dev.astype(NP_BF16).astype(np.float64) * cscale[:, None]
    pred_raw_x = pos_dev.astype(np.float64) * cscale[:, None]

    CRS = cand.transpose(1, 0, 2, 3)                # [512, 8cores, NSLOT, 8]
    owner = labels_p // L_LOCAL
    for i in rows:
        rt = i // 128
        for c in range(C):
            s = inv_order[c]
            kind = slab_kind[rt][s]
            sl = slab_slot[rt, s]
            if kind == "exp":
                pv = pred_exp[i, c]
            elif kind == "copy":
                pv = pred_raw_b[i, c]
            else:
                pv = pred_raw_x[i, c]
            vals = CRS[i, owner[i], sl]
            d = np.abs(vals - pv)
            j = np.argmin(d)
            if d[j] < 2.5e-4 + 5e-3 * abs(pv):
                CRS[i, owner[i], sl, j] = -np.inf

    CR = CRS.reshape(N, NCORES * NSLOT * CAND_PER_S)
    nc_tot = CR.shape[1]
    part = np.partition(CR, nc_tot - K, axis=1)[:, -K:]  # top-50 values
    t50 = part.min(axis=1)

    # at-risk check: each chunk's 8th-largest candidate should be <= t50
    # (sound certificate for 'direct' chunks, heuristic for folded chunks)
    slab8 = np.where(act[None], cand[:, :, :, CAND_PER_S - 1], -np.inf)
    if _FORCE_FALLBACK:
        bad = rows
    else:
        bad = np.where(slab8.max(axis=(0, 2)) > t50)[0]
    global _LAST_FALLBACKS
    _LAST_FALLBACKS = len(bad)
    if len(bad):
        sims_bad = fn[bad] @ cen.T                              # [nbad, 64000]
        for bi, i in enumerate(bad):
            srow = sims_bad[bi]
            srow[C * labels_p[i] : C * labels_p[i] + C] = -np.inf
            part[i] = np.sort(srow)[-K:]

    z = np.concatenate([pos, part], axis=1) * INV_T             # [512, 58]
    mz = z.max(axis=1)
    lse_inter = np.log(np.exp(z - mz[:, None]).sum(axis=1)) + mz
    loss_inter_i = lse_inter - INV_T * pos.mean(axis=1)

    # ---- per-camera means, summed ----
    cnt = np.bincount(cams_p, minlength=C).astype(np.float64)
    s_intra = np.bincount(cams_p, weights=loss_intra_i, minlength=C)
    s_inter = np.bincount(cams_p, weights=loss_inter_i, minlength=C)
    safe = np.maximum(cnt, 1.0)
    li = np.sum(np.where(cnt > 0, s_intra / safe, 0.0))
    le = LW * np.sum(np.where(cnt > 0, s_inter / safe, 0.0))
    return np.array([li, le], dtype=np.float32)


def _prepare(feats, indexes, label_table, cam_table, centers):
    feats = np.asarray(feats, dtype=np.float32)
    indexes = np.asarray(indexes)
    label_table = np.asarray(label_table)
    cam_table = np.asarray(cam_table)
    centers = np.asarray(centers, dtype=np.float32)

    labels = np.asarray(label_table[indexes], dtype=np.int64)
    cams = np.asarray(cam_table[indexes], dtype=np.int64)

    # permute rows so camera groups are contiguous, ordered so most 128-row
    # tiles span only ~2 cameras (fewer intra exp instructions)
    sizes = np.bincount(cams, minlength=C)
    order = _pair_order(sizes)
    perm = np.concatenate([np.where(cams == c)[0] for c in order])
    feats_p = np.ascontiguousarray(feats[perm])
    labels_p = labels[perm]
    cams_p = cams[perm]
    inv_order = np.argsort(np.asarray(order))  # camera -> slab index
    slabs_p = inv_order[cams_p]
    # tile_cams in SLAB indices: consecutive runs by construction
    tile_cams = tuple(
        tuple(dict.fromkeys(slabs_p[128 * rt : 128 * (rt + 1)].tolist()))
        for rt in range(RT)
    )

    # per-core centers, slab s = camera order[s], 24-col pad per slab,
    # pre-scaled, transposed to [128, 2, PL] (partition=feat_lo, j=feat_hi)
    np_mm = NP_FP8 if MM == "fp8dr" else NP_BF16
    by_cam = centers.reshape(L, C, D)
    cenT_shards = []
    for k in range(NCORES):
        X = by_cam[k * L_LOCAL : (k + 1) * L_LOCAL]             # [1000, 8, 256]
        CP = np.zeros((C, SLABW, D), dtype=np.float32)
        for s in range(C):
            CP[s, 0:SW] = X[:, order[s], :]
        CP = (CEN_SCALE * CP).reshape(PL, 2, 128)
        cenT_shards.append(
            np.ascontiguousarray(CP.transpose(2, 1, 0), dtype=np_mm)
        )
    return centers, tile_cams, feats_p, labels_p, cams_p, cenT_shards, order


def kernel(feats, indexes, label_table, cam_table, centers):
    centers, tile_cams, feats_p, labels_p, cams_p, cenT_shards, order = _prepare(
        feats, indexes, label_table, cam_table, centers
    )
    nc = _build_program(tile_cams)
    runner = _get_runner(nc)
    runner.put_inputs(_make_in_maps(cenT_shards, feats_p))
    results = runner.execute()
    return _host_finish(
        results, feats_p, labels_p, cams_p, centers, tile_cams, order
    )
